# revision 1
# baseline (speedup 1.0000x reference)
"""Trainium2 Bass kernel for nn_DeepLinear (784->10 linear + BN, 62x(10->10 linear + BN), 10->10 linear).

Math: BN output has exact per-column batch mean beta, so every layer past the first
acts linearly on the *centered* activations. The whole net collapses to:
    h  = x @ W0.T                      (heavy, on device, data-parallel over batch)
    mu = mean(h), C = cov(h)           (global batch moments; partial moments per core,
                                        combined on host = the sync-BN all-reduce)
    T, r = 62-layer chain of 10x10 covariance algebra (tiny, host, float64)
    out = (h - mu) @ T + r             (light, on device)

The big matmul runs as an exact fp16 hi/lo split: x = xh + xl, W0 = Wh + Wl in fp16,
accumulating all four cross terms in fp32 PSUM -> bit-accuracy ~fp32 at 1 cycle/row.
"""

import numpy as np

EPS = 1e-5
B = 65536
D = 784
NCORES = 8
BC = B // NCORES          # 8192 rows per core
KP = 112                  # contraction chunk partitions (7 * 112 = 784)
KC = 7                    # contraction chunks
CB = 512                  # batch columns per DMA chunk
NT = 512                  # batch columns per matmul
NBLK = BC // 128          # 64 blocks of 128 rows per core

_cache = {}
STAGE1_CHUNKS = [512] * 16


def _f32(dt_mod):
    return dt_mod.float32


def _build_stage1(chunks=None, xbufs=4, psh_bufs=3, ps2_bufs=2, pst_bufs=2, lo_bufs=3, paired=False):
    import concourse.bacc as bacc
    import concourse.mybir as mybir
    from concourse.tile import TileContext
    from concourse.masks import make_identity

    F16 = mybir.dt.float16
    F32 = mybir.dt.float32

    nc = bacc.Bacc("TRN2", target_bir_lowering=False, debug=False, num_devices=NCORES)
    xh = nc.dram_tensor("xh", [D * BC], F16, kind="ExternalInput")
    F8 = mybir.dt.float8e4
    xl = nc.dram_tensor("xl", [D * BC], F8, kind="ExternalInput")
    wp = nc.dram_tensor("wp", [KP, KC * 84], F16, kind="ExternalInput")
    F8E4 = mybir.dt.float8e4
    wp8 = nc.dram_tensor("wp8", [KP, KC * 16], F8E4, kind="ExternalInput")
    hb = nc.dram_tensor("hb", [128, NBLK * 10], F32, kind="ExternalOutput")
    mom = nc.dram_tensor("mom", [10, 11], F32, kind="ExternalOutput")

    with TileContext(nc) as tc:
        with (
            tc.tile_pool(name="const", bufs=1) as cpool,
            tc.tile_pool(name="xs", bufs=xbufs) as xpool,
            tc.tile_pool(name="hts", bufs=1) as hpool,
            tc.tile_pool(name="ps_h", bufs=psh_bufs, space="PSUM") as ps_h,
            tc.tile_pool(name="ps_2", bufs=ps2_bufs, space="PSUM") as ps_2,
            tc.tile_pool(name="ps_t", bufs=pst_bufs, space="PSUM") as ps_t,
            tc.tile_pool(name="ps_s", bufs=1, space="PSUM") as ps_s,
        ):
            wp_sb = cpool.tile([KP, KC * 84], F16, name="wp_sb")
            nc.sync.dma_start(wp_sb[:], wp[:])
            wp8_sb = cpool.tile([KP, KC, 16], F8E4, name="wp8_sb")
            nc.sync.dma_start(
                wp8_sb[:], wp8[:].rearrange("p (k m) -> p k m", k=KC)
            )
            ident = cpool.tile([10, 10], F32, name="ident")
            make_identity(nc, ident[:])

            ht_sb = hpool.tile([10, BC], F32, name="ht_sb")
            hn_sb = hpool.tile([128, NBLK * 10], F32, name="hn_sb")
            s1p = hpool.tile([10, 24], F32, name="s1p")
            mom_sb = hpool.tile([10, 11], F32, name="mom_sb")

            ps_S = ps_s.tile([10, 10], F32, name="ps_S")

            nblk_per_tile = NT // 128  # 4
            blk = 0
            pending = None

            def emit_sblock(b0, nb):
                # nb transposes share one psum tile, drained by one copy
                pt = ps_t.tile([128, 4 * 10], F32, tag="pt", name="pt")
                for bb in range(nb):
                    nc.tensor.transpose(
                        pt[:, bb * 10:(bb + 1) * 10],
                        ht_sb[:, (b0 + bb) * 128:(b0 + bb + 1) * 128],
                        ident[:],
                    )
                nc.vector.tensor_copy(
                    hn_sb[:, b0 * 10:(b0 + nb) * 10], pt[:, 0:nb * 10]
                )
                for bb in range(nb):
                    b2 = b0 + bb
                    nc.tensor.matmul(
                        ps_S[:],
                        hn_sb[:, b2 * 10:(b2 + 1) * 10],
                        hn_sb[:, b2 * 10:(b2 + 1) * 10],
                        start=(b2 == 0),
                        stop=(b2 == NBLK - 1),
                    )
            # ramp-up / ramp-down chunk widths: small first chunk lets PE start
            # early; small last chunks shrink the post-DMA tail
            CHUNKS = chunks or STAGE1_CHUNKS
            assert sum(CHUNKS) == BC
            off = 0
            pos = 0
            for ob, W in enumerate(CHUNKS):
                # one DMA per plane per chunk: dest covers all 7 k-slabs
                xh_t = xpool.tile([KP, KC, CB], F16, tag="xh", name="xh_t")
                nc.sync.dma_start(
                    xh_t[:, :, 0:W],
                    xh[pos:pos + KP * KC * W].rearrange(
                        "(p k w) -> p k w", p=KP, k=KC
                    ),
                )
                xl_t = xpool.tile([KP, KC, CB], F8, tag="xl", name="xl_t")
                nc.sync.dma_start(
                    xl_t[:, :, 0:W],
                    xl[pos:pos + KP * KC * W].rearrange(
                        "(p k w) -> p k w", p=KP, k=KC
                    ),
                )
                pos += KP * KC * W
                for j in range((W + NT - 1) // NT):
                    n = min(NT, W - j * NT)
                    ps = ps_h.tile([128, NT], F32, tag="ps", name="ps")
                    ps2 = ps_2.tile([10, NT], F32, tag="ps2", name="ps2")
                    # hi pass: fp16, psum[0:10] = xh@Wh, psum[32:42] = xh@Wl*2^6
                    for k in range(KC):
                        nc.tensor.matmul(
                            ps[0:42, 0:n],
                            wp_sb[:, k * 84:k * 84 + 42],
                            xh_t[:, k, j * NT:j * NT + n],
                            start=(k == 0),
                            stop=(k == KC - 1),
                        )
                    # lo pass: fp8e4 DoubleRow pairs two 112-row chunks per
                    # matmul; psum[64:74] = e@W * 2^16
                    for kp in range(0, KC - 1, 2):
                        nc.tensor.matmul(
                            ps2[:, 0:n],
                            wp8_sb[:, kp:kp + 2, 0:10],
                            xl_t[:, kp:kp + 2, j * NT:j * NT + n],
                            start=(kp == 0),
                            stop=False,
                            perf_mode=mybir.MatmulPerfMode.DoubleRow,
                        )
                    nc.tensor.matmul(
                        ps2[:, 0:n],
                        wp8_sb[:, KC - 1, 0:10],
                        xl_t[:, KC - 1, j * NT:j * NT + n],
                        start=False,
                        stop=True,
                    )
                    col0 = off + j * NT
                    lo_t = hpool.tile([10, NT], F32, tag="lo", bufs=lo_bufs, name="lo_t")
                    nc.scalar.activation(
                        lo_t[:, 0:n], ps[32:42, 0:n],
                        mybir.ActivationFunctionType.Copy, scale=2.0 ** -6,
                    )
                    lo_u = hpool.tile([10, NT], F32, tag="lou", bufs=lo_bufs, name="lo_u")
                    nc.scalar.activation(
                        lo_u[:, 0:n], ps2[:, 0:n],
                        mybir.ActivationFunctionType.Copy, scale=2.0 ** -16,
                    )
                    nc.vector.tensor_add(lo_t[:, 0:n], lo_t[:, 0:n], lo_u[:, 0:n])
                    nc.vector.tensor_add(
                        ht_sb[:, col0:col0 + n], ps[0:10, 0:n], lo_t[:, 0:n]
                    )
                    if pending is not None:
                        emit_sblock(*pending)
                    pending = (blk, n // 128)
                    blk += n // 128
                nc.vector.reduce_sum(
                    s1p[:, ob:ob + 1],
                    ht_sb[:, off:off + W],
                    axis=mybir.AxisListType.X,
                )
                off += W
            if pending is not None:
                emit_sblock(*pending)
            nc.vector.reduce_sum(
                mom_sb[:, 0:1], s1p[:, 0:len(CHUNKS)],
                axis=mybir.AxisListType.X,
            )
            nc.vector.tensor_copy(mom_sb[:, 1:11], ps_S[:])
            # 3-way split: only the last 4 blocks' piece sits on the tail
            c1 = NBLK * 10 // 2            # blocks 0-31
            c2 = (NBLK - 4) * 10           # blocks 32-59
            nc.sync.dma_start(hb[:, 0:c1], hn_sb[:, 0:c1])
            nc.sync.dma_start(hb[:, c1:c2], hn_sb[:, c1:c2])
            nc.sync.dma_start(hb[:, c2:], hn_sb[:, c2:])
            nc.sync.dma_start(mom[:], mom_sb[:])
    nc.finalize()
    return nc


def _build_stage2():
    import concourse.bacc as bacc
    import concourse.mybir as mybir
    from concourse.tile import TileContext

    F16 = mybir.dt.float16
    F32 = mybir.dt.float32

    nc = bacc.Bacc("TRN2", target_bir_lowering=False, debug=False, num_devices=NCORES)
    hp = nc.dram_tensor("hp", [33, BC], F16, kind="ExternalInput")
    tm = nc.dram_tensor("tm", [33, 10], F16, kind="ExternalInput")
    ob = nc.dram_tensor("ob", [128, NBLK * 10], F32, kind="ExternalOutput")

    with TileContext(nc) as tc:
        with (
            tc.tile_pool(name="sb", bufs=1) as sb,
            tc.tile_pool(name="ps", bufs=4, space="PSUM") as psp,
        ):
            tm_sb = sb.tile([33, 10], F16, name="tm_sb")
            nc.sync.dma_start(tm_sb[:], tm[:])
            hp_sb = sb.tile([33, BC], F16, name="hp_sb")
            half = BC // 2
            for p in range(2):
                nc.sync.dma_start(
                    hp_sb[:, p * half:(p + 1) * half],
                    hp[:, p * half:(p + 1) * half],
                )
            ob_sb = sb.tile([128, NBLK * 10], F32, name="ob_sb")
            GRP = 16
            for g0 in range(NBLK // GRP):
                ps = psp.tile([128, GRP * 10], F32, tag="ps", name="ps")
                for bb in range(GRP):
                    b = g0 * GRP + bb
                    nc.tensor.matmul(
                        ps[:, bb * 10:(bb + 1) * 10],
                        hp_sb[:, b * 128:(b + 1) * 128],
                        tm_sb[:],
                        start=True,
                        stop=True,
                    )
                nc.vector.tensor_copy(
                    ob_sb[:, g0 * GRP * 10:(g0 + 1) * GRP * 10], ps[:]
                )
            nc.sync.dma_start(ob[:], ob_sb[:])
    nc.finalize()
    return nc


def _chain_host(s1, S, W0, b0, g0, beta0, Ws, bs, gs, betas, Wf, bf):
    """Collapse BN chain on global moments of h = x@W0.T (no bias). float64.
    Returns Tmat [10,10], r [10] with out = h @ Tmat + r."""
    m = s1.astype(np.float64) / B
    C = S.astype(np.float64) / B - np.outer(m, m)
    g0 = g0.astype(np.float64)
    var0 = np.diag(C).copy()
    A = np.diag(g0 / np.sqrt(var0 + EPS))
    d = beta0.astype(np.float64).copy()
    Ws64 = Ws.astype(np.float64)
    gs64 = gs.astype(np.float64)
    betas64 = betas.astype(np.float64)
    for k in range(Ws64.shape[0]):
        Ak = A @ Ws64[k].T
        var = np.einsum("ij,ik,kj->j", Ak, C, Ak)
        A = Ak * (gs64[k] / np.sqrt(var + EPS))[None, :]
        d = betas64[k].copy()
    Tmat = A @ Wf.astype(np.float64).T
    r = d @ Wf.astype(np.float64).T + bf.astype(np.float64)
    # fold bias b0 and centering: out = (h + b0 - (m + b0)) @ Tmat + r
    return Tmat, (r - m @ Tmat)


def _split16(a):
    hi = a.astype(np.float16)
    lo = (a.astype(np.float32) - hi.astype(np.float32)).astype(np.float16)
    return hi, lo


def kernel(**inputs):
    from concourse.bass_utils import run_bass_kernel_spmd

    inputs = {k: np.asarray(v, dtype=np.float32) for k, v in inputs.items()}
    x = inputs["x"]
    W0 = inputs["W0"]

    if "nc1" not in _cache:
        _cache["nc1"] = _build_stage1(chunks=STAGE1_CHUNKS)
    if "nc2" not in _cache:
        _cache["nc2"] = _build_stage2()

    # ---- host marshalling for stage 1 ----
    import ml_dtypes
    F8 = ml_dtypes.float8_e4m3
    xh = x.astype(np.float16)                 # [B, D]
    e = x - xh.astype(np.float32)             # exact residual
    xl8 = (e * 4096.0).astype(F8)             # fp8e3, scale 2^12 (|.| <= ~11)
    xh_t = xh.T                               # [D, B] strided views
    xl_t = xl8.T
    W0h, W0l = _split16(W0)                   # [10, D]
    # hi-pass stationary (cols k*84..k*84+41):  W0h at +0..9, W0l*2^6 at +32..41
    # lo-pass stationary (cols k*84+42..+83):   zeros at +0..9, W0h*2^-6 at +32..41
    # psum[0:10] = xh@W0h ; psum[32:42] = (xh@W0l + e@W0h) * 2^6
    # ht = psum[0:10] + 2^-6 * psum[32:42]  (ACT applies the 2^-6 on its copy)
    wph = W0h.T.reshape(KC, KP, 10).transpose(1, 0, 2)   # [112, 7, 10]
    wpl = (W0l.astype(np.float32) * 2.0 ** 6).astype(np.float16)
    wpl = wpl.T.reshape(KC, KP, 10).transpose(1, 0, 2)
    wph_dn = (W0h.astype(np.float32) * 2.0 ** -6).astype(np.float16)
    wph_dn = wph_dn.T.reshape(KC, KP, 10).transpose(1, 0, 2)
    wp = np.zeros((KP, KC, 84), dtype=np.float16)
    wp[:, :, 0:10] = wph
    wp[:, :, 32:42] = wpl
    wp[:, :, 74:84] = wph_dn
    wp = np.ascontiguousarray(wp.reshape(KP, KC * 84))
    # fp8e4 lo-pass stationary: full W * 2^4 (psum term lands at scale 2^16)
    w8 = (W0.astype(np.float32) * 2.0 ** 4).astype(F8)
    w8 = w8.T.reshape(KC, KP, 10).transpose(1, 0, 2)     # [112, 7, 10]
    wp8 = np.zeros((KP, KC, 16), dtype=F8)
    wp8[:, :, 0:10] = w8
    wp8 = np.ascontiguousarray(wp8.reshape(KP, KC * 16))

    CHUNKS = STAGE1_CHUNKS
    in1 = []
    for c in range(NCORES):
        sl = slice(c * BC, (c + 1) * BC)
        xhc = np.ascontiguousarray(xh_t[:, sl])      # [784, 8192] fp16
        xlc = np.ascontiguousarray(xl_t[:, sl])      # [784, 8192] fp8
        hblob = np.empty(D * BC, dtype=np.float16)
        lblob = np.empty(D * BC, dtype=xlc.dtype)
        h3 = xhc.reshape(KC, KP, BC)
        l3 = xlc.reshape(KC, KP, BC)
        pos = 0
        off = 0
        for W in CHUNKS:
            n = KP * KC * W
            hblob[pos:pos + n] = h3[:, :, off:off + W].transpose(1, 0, 2).ravel()
            lblob[pos:pos + n] = l3[:, :, off:off + W].transpose(1, 0, 2).ravel()
            pos += n
            off += W
        in1.append({"xh": hblob, "xl": lblob, "wp": wp, "wp8": wp8})
    res1 = run_bass_kernel_spmd(_cache["nc1"], in1, core_ids=list(range(NCORES)))

    # ---- gather moments, run the tiny chain on host ----
    s1 = np.zeros(10, dtype=np.float64)
    S = np.zeros((10, 10), dtype=np.float64)
    h_parts = []
    for c in range(NCORES):
        mom = np.asarray(res1.results[c]["mom"], dtype=np.float64)
        s1 += mom[:, 0]
        S += mom[:, 1:11]
        hbc = np.asarray(res1.results[c]["hb"])          # [128, 640]
        h_parts.append(hbc.reshape(128, NBLK, 10).transpose(1, 0, 2).reshape(BC, 10))
    h = np.concatenate(h_parts, axis=0)                   # [B, 10] fp32

    Tmat, r = _chain_host(
        s1, S,
        W0, inputs["b0"], inputs["g0"], inputs["beta0"],
        inputs["Ws"], inputs["bs"], inputs["gs"], inputs["betas"],
        inputs["Wf"], inputs["bf"],
    )

    # ---- host marshalling for stage 2 ----
    Tb = np.concatenate([Tmat, r[None, :]], axis=0).astype(np.float32)  # [11, 10]
    Tbh, Tbl = _split16(Tb)
    # K=33 pairing: [hth;1]@Tbh + [htl;0]@Tbh + [hth;1]@Tbl
    tmv = np.concatenate([Tbh, Tbh, Tbl], axis=0)        # [33, 10] fp16

    ht = h.T                                              # [10, B] fp32 view
    hth, htl = _split16(ht)                               # [10, B] fp16
    in2 = []
    for c in range(NCORES):
        sl = slice(c * BC, (c + 1) * BC)
        hpc = np.zeros((33, BC), dtype=np.float16)
        hpc[0:10] = hth[:, sl]
        hpc[10, :] = 1.0
        hpc[11:21] = htl[:, sl]
        hpc[22:32] = hth[:, sl]
        hpc[32, :] = 1.0
        in2.append({"hp": hpc, "tm": tmv})
    res2 = run_bass_kernel_spmd(_cache["nc2"], in2, core_ids=list(range(NCORES)))

    out_parts = []
    for c in range(NCORES):
        obc = np.asarray(res2.results[c]["ob"])           # [128, 640]
        out_parts.append(obc.reshape(128, NBLK, 10).transpose(1, 0, 2).reshape(BC, 10))
    return np.ascontiguousarray(np.concatenate(out_parts, axis=0))

